# revision 3
# baseline (speedup 1.0000x reference)
"""Trainium2 Bass kernel for the MoE-routed 3-layer LoRA MLP.

Strategy: pure data-parallel over the batch (16384 rows -> 2048 per core,
8 cores, no collectives). On-device layout is feature-major (transposed):
activations live as [features, batch] so every matmul contracts over the
partition dimension without any on-device transposes. All matmul operands
are bf16 (PSUM accumulation is f32); the tiny domain-routing network runs
in f32 on device and is folded into a per-domain gamma = zeta * alpha
table, gathered to per-token scale rows via a one-hot matmul.

Per core the three layers are fused column-by-column (4 columns of 512
tokens): weights for all layers stay SBUF-resident; h1/h2 never touch DRAM.
"""

import json

import numpy as np
import ml_dtypes

import concourse.bass as bass
import concourse.tile as tile
from concourse import mybir
from concourse.bass_utils import run_bass_kernel_spmd

F32 = mybir.dt.float32
BF16 = mybir.dt.bfloat16
AF = mybir.ActivationFunctionType
ALU = mybir.AluOpType
AX = mybir.AxisListType

N_CORES = 8
BSZ, D0, D1, D2, D3 = 16384, 2048, 2048, 1024, 512
E, RK, M, H, L = 4, 8, 8, 64, 3
B_LOC = BSZ // N_CORES  # 2048
NT = 4                  # batch columns per core
NB = B_LOC // NT        # 512
BF_NP = ml_dtypes.bfloat16


# ---------------------------------------------------------------------------
# BIR post-pass: this container's walrus rejects instructions carrying more
# than one semaphore wait; split extras onto preceding same-engine NoOps
# (the engine sequencer processes waits before the instruction, so this is
# semantics-preserving).
# ---------------------------------------------------------------------------
def _split_waits(bir, max_waits=1):
    counter = [0]

    def fix_block(bb):
        new_instructions = []
        for ins in bb.get("instructions", []):
            si = ins.get("sync_info") or {}
            waits = si.get("on_wait") or []
            if len(waits) > max_waits:
                head, tail = waits[:-max_waits], waits[-max_waits:]
                for i in range(0, len(head), max_waits):
                    counter[0] += 1
                    new_instructions.append(
                        {
                            "engine": ins["engine"],
                            "ins": [],
                            "name": f"I-waitsplit-{counter[0]}",
                            "opcode": "Drain",
                            "outs": [],
                            "sync_info": {
                                "on_update": [],
                                "on_wait": head[i : i + max_waits],
                            },
                        }
                    )
                si = dict(si)
                si["on_wait"] = tail
                ins = dict(ins)
                ins["sync_info"] = si
            new_instructions.append(ins)
        if "instructions" in bb:
            bb["instructions"] = new_instructions
        for inner in bb.get("blocks", []):
            fix_block(inner)

    for fn in bir.get("functions", []):
        for bb in fn.get("blocks", []):
            fix_block(bb)
    return bir


def _patch_bass_json(nc):
    orig = nc.to_json_bytes

    def wrapped(*a, **k):
        return json.dumps(_split_waits(json.loads(orig(*a, **k)))).encode()

    nc.to_json_bytes = wrapped


# ---------------------------------------------------------------------------
# Routing: compute gexp [8, 96] f32 where
#   gexp[m, l*32 + e*8 + r] = zeta_agg[m, l] * alpha_agg[m, l, e]
# ---------------------------------------------------------------------------
def _build_routing(nc, const, small, psum, dram):
    ML = M * L
    rin = nc.dram_tensor("rin", [2 * H, ML], F32, kind="ExternalInput")
    wi1t = nc.dram_tensor("wi1t", [2 * H, H], F32, kind="ExternalInput")
    wa1t = nc.dram_tensor("wa1t", [2 * H, H], F32, kind="ExternalInput")
    bi1v = nc.dram_tensor("bi1v", [H], F32, kind="ExternalInput")
    ba1v = nc.dram_tensor("ba1v", [H], F32, kind="ExternalInput")
    wi2b = nc.dram_tensor("wi2b", [H + 1, 1], F32, kind="ExternalInput")
    wa2b = nc.dram_tensor("wa2b", [H + 1, E], F32, kind="ExternalInput")
    gatet = nc.dram_tensor("gatet", [M, M], F32, kind="ExternalInput")
    rbt = nc.dram_tensor("rbt", [M, M], F32, kind="ExternalInput")

    rin_s = const.tile([2 * H, ML], F32, tag="rin")
    wi1t_s = const.tile([2 * H, H], F32, tag="wi1t")
    wa1t_s = const.tile([2 * H, H], F32, tag="wa1t")
    bi1_s = const.tile([H, 1], F32, tag="bi1")
    ba1_s = const.tile([H, 1], F32, tag="ba1")
    wi2b_s = const.tile([H + 1, 1], F32, tag="wi2b")
    wa2b_s = const.tile([H + 1, E], F32, tag="wa2b")
    gatet_s = const.tile([M, M], F32, tag="gatet")
    rbt_s = const.tile([M, M], F32, tag="rbt")
    for t, d in [
        (rin_s, rin), (wi1t_s, wi1t), (wa1t_s, wa1t),
        (wi2b_s, wi2b), (wa2b_s, wa2b), (gatet_s, gatet), (rbt_s, rbt),
    ]:
        nc.sync.dma_start(out=t[:], in_=d[:])
    nc.sync.dma_start(out=bi1_s[:], in_=bi1v.rearrange("(h one) -> h one", one=1))
    nc.sync.dma_start(out=ba1_s[:], in_=ba1v.rearrange("(h one) -> h one", one=1))

    # router hidden layers, with an extra ones-row to fold the output bias
    hz_ext = small.tile([H + 1, ML], F32, tag="hz")
    ha_ext = small.tile([H + 1, ML], F32, tag="ha")
    for wt, bt, ext in [(wi1t_s, bi1_s, hz_ext), (wa1t_s, ba1_s, ha_ext)]:
        ps = psum.tile([H, ML], F32, tag="rpsum")
        nc.tensor.matmul(ps[:], wt[:], rin_s[:], start=True, stop=True)
        nc.scalar.activation(ext[0:H, :], ps[:], AF.Relu, bias=bt[:])
        nc.vector.memset(ext[H : H + 1, :], 1.0)

    # zeta logits [24,1] -> [8,3] via DRAM bounce
    zps = psum.tile([ML, 1], F32, tag="rpsum")
    nc.tensor.matmul(zps[:], hz_ext[:], wi2b_s[:], start=True, stop=True)
    z24 = small.tile([ML, 1], F32, tag="z24")
    nc.vector.tensor_copy(z24[:], zps[:])
    zdram = dram.tile([ML, 1], F32, tag="zdram")
    nc.sync.dma_start(out=zdram[:], in_=z24[:])
    zl = small.tile([M, L], F32, tag="zl")
    nc.sync.dma_start(out=zl[:], in_=zdram.rearrange("(m l) one -> m (l one)", m=M))

    # alpha logits [24,4]
    aps = psum.tile([ML, E], F32, tag="rpsum")
    nc.tensor.matmul(aps[:], ha_ext[:], wa2b_s[:], start=True, stop=True)
    al = small.tile([ML, E], F32, tag="al")
    nc.vector.tensor_copy(al[:], aps[:])

    # zeta sparse softmax over L=3, keep top-2 (drop the min)
    zneg = small.tile([M, L], F32, tag="zneg")
    nc.vector.tensor_scalar_mul(zneg[:], zl[:], -1.0)
    zmin = small.tile([M, 1], F32, tag="zmin")
    nc.vector.reduce_max(zmin[:], zneg[:], axis=AX.X)
    nc.vector.tensor_scalar_mul(zmin[:], zmin[:], -1.0)
    zmax = small.tile([M, 1], F32, tag="zmax")
    nc.vector.reduce_max(zmax[:], zl[:], axis=AX.X)
    zmaxn = small.tile([M, 1], F32, tag="zmaxn")
    nc.vector.tensor_scalar_mul(zmaxn[:], zmax[:], -1.0)
    ze = small.tile([M, L], F32, tag="ze")
    nc.scalar.activation(ze[:], zl[:], AF.Exp, bias=zmaxn[:])
    zmask = small.tile([M, L], F32, tag="zmask")
    nc.vector.tensor_scalar(zmask[:], zl[:], zmin[:], None, ALU.is_gt)
    nc.vector.tensor_mul(ze[:], ze[:], zmask[:])
    zs = small.tile([M, 1], F32, tag="zs")
    nc.vector.reduce_sum(zs[:], ze[:], axis=AX.X)
    zrs = small.tile([M, 1], F32, tag="zrs")
    nc.vector.reciprocal(zrs[:], zs[:])
    zeta_all = small.tile([M, L], F32, tag="zeta_all")
    nc.vector.tensor_scalar_mul(zeta_all[:], ze[:], zrs[:])

    # alpha sparse softmax over E=4, keep top-2 (threshold = 2nd max)
    m1 = small.tile([ML, 1], F32, tag="m1")
    nc.vector.reduce_max(m1[:], al[:], axis=AX.X)
    m1n = small.tile([ML, 1], F32, tag="m1n")
    nc.vector.tensor_scalar_mul(m1n[:], m1[:], -1.0)
    meq = small.tile([ML, E], F32, tag="meq")
    nc.vector.tensor_scalar(meq[:], al[:], m1[:], None, ALU.is_equal)
    nc.vector.tensor_scalar_mul(meq[:], meq[:], 1e30)
    v2 = small.tile([ML, E], F32, tag="v2")
    nc.vector.tensor_sub(v2[:], al[:], meq[:])
    m2 = small.tile([ML, 1], F32, tag="m2")
    nc.vector.reduce_max(m2[:], v2[:], axis=AX.X)
    keep = small.tile([ML, E], F32, tag="keep")
    nc.vector.tensor_scalar(keep[:], al[:], m2[:], None, ALU.is_ge)
    ae = small.tile([ML, E], F32, tag="ae")
    nc.scalar.activation(ae[:], al[:], AF.Exp, bias=m1n[:])
    nc.vector.tensor_mul(ae[:], ae[:], keep[:])
    as_ = small.tile([ML, 1], F32, tag="as_")
    nc.vector.reduce_sum(as_[:], ae[:], axis=AX.X)
    ars = small.tile([ML, 1], F32, tag="ars")
    nc.vector.reciprocal(ars[:], as_[:])
    alpha_all = small.tile([ML, E], F32, tag="alpha_all")
    nc.vector.tensor_scalar_mul(alpha_all[:], ae[:], ars[:])

    # [24,4] -> [8,12] via DRAM bounce
    adram = dram.tile([ML, E], F32, tag="adram")
    nc.sync.dma_start(out=adram[:], in_=alpha_all[:])
    alpha8 = small.tile([M, L * E], F32, tag="alpha8")
    nc.sync.dma_start(out=alpha8[:], in_=adram.rearrange("(m l) e -> m (l e)", m=M))

    # RuT[n,m] = softplus(gate[m,n]) * Rb[m,n]   (softplus = ln(1+exp))
    rut = small.tile([M, M], F32, tag="rut")
    nc.scalar.activation(rut[:], gatet_s[:], AF.Exp)
    nc.vector.tensor_scalar_add(rut[:], rut[:], 1.0)
    nc.scalar.activation(rut[:], rut[:], AF.Ln)
    nc.vector.tensor_mul(rut[:], rut[:], rbt_s[:])

    # aggregate [zeta(3) | alpha(12) | ones(1)] through RuT, then normalize
    W16 = L + L * E + 1
    agg_rhs = small.tile([M, W16], F32, tag="agg_rhs")
    nc.vector.tensor_copy(agg_rhs[:, 0:L], zeta_all[:])
    nc.vector.tensor_copy(agg_rhs[:, L : L + L * E], alpha8[:])
    nc.vector.memset(agg_rhs[:, W16 - 1 : W16], 1.0)
    agg_ps = psum.tile([M, W16], F32, tag="rpsum")
    nc.tensor.matmul(agg_ps[:], rut[:], agg_rhs[:], start=True, stop=True)
    rsum = small.tile([M, 1], F32, tag="rsum")
    nc.vector.tensor_scalar_max(rsum[:], agg_ps[:, W16 - 1 : W16], 1e-12)
    rrs = small.tile([M, 1], F32, tag="rrs")
    nc.vector.reciprocal(rrs[:], rsum[:])
    table = small.tile([M, L + L * E], F32, tag="table")
    nc.vector.tensor_scalar_mul(table[:], agg_ps[:, 0 : L + L * E], rrs[:])

    # gamma12[m, l*4+e] = zeta[m,l] * alpha[m, l*4+e]
    zexp = small.tile([M, L * E], F32, tag="zexp")
    zview = zexp.rearrange("p (l e) -> p l e", e=E)
    for e in range(E):
        nc.vector.tensor_copy(zview[:, :, e], table[:, 0:L])
    gamma12 = small.tile([M, L * E], F32, tag="gamma12")
    nc.vector.tensor_mul(gamma12[:], table[:, L : L + L * E], zexp[:])

    # expand over rank r: gexp[:, l*32 + e*8 + r] = gamma12[:, l*4+e]
    gexp = small.tile([M, L * E * RK], F32, tag="gexp")
    gview = gexp.rearrange("p (le r) -> p le r", r=RK)
    for r in range(RK):
        nc.vector.tensor_copy(gview[:, :, r], gamma12[:])
    return gexp


# ---------------------------------------------------------------------------
# Full per-core graph
# ---------------------------------------------------------------------------
def _build(nc):
    DIMS = [(D0, D1), (D1, D2), (D2, D3)]

    xt = nc.dram_tensor("xt", [D0, B_LOC], BF16, kind="ExternalInput")
    onehot = nc.dram_tensor("onehot", [M, B_LOC], F32, kind="ExternalInput")
    wts = [
        nc.dram_tensor(f"w{l + 1}t", [i, o], BF16, kind="ExternalInput")
        for l, (i, o) in enumerate(DIMS)
    ]
    ats = [
        nc.dram_tensor(f"a{l + 1}t", [i, E * RK], BF16, kind="ExternalInput")
        for l, (i, _) in enumerate(DIMS)
    ]
    lbs = [
        nc.dram_tensor(f"lb{l + 1}", [E * RK, o], BF16, kind="ExternalInput")
        for l, (_, o) in enumerate(DIMS)
    ]
    biases = [
        nc.dram_tensor(f"bias{l + 1}", [o], F32, kind="ExternalInput")
        for l, (_, o) in enumerate(DIMS)
    ]
    out_d = nc.dram_tensor("out", [D3, B_LOC], F32, kind="ExternalOutput")

    with tile.TileContext(nc) as tc:
        with (
            tc.tile_pool(name="const", bufs=1) as const,
            tc.tile_pool(name="small", bufs=1) as small,
            tc.tile_pool(name="rpsum", bufs=2, space="PSUM") as rpsum,
            tc.tile_pool(name="dram", bufs=1, space="DRAM") as dram,
            tc.tile_pool(name="wpool", bufs=1) as wpool,
            tc.tile_pool(name="gpool", bufs=1) as gpool,
            tc.tile_pool(name="onp", bufs=2) as onp,
            tc.tile_pool(name="xcol", bufs=18) as xcolp,
            tc.tile_pool(name="h1", bufs=18) as h1p,
            tc.tile_pool(name="h2", bufs=10) as h2p,
            tc.tile_pool(name="oc", bufs=4) as ocp,
            tc.tile_pool(name="tw", bufs=3) as twp,
            tc.tile_pool(name="mmps", bufs=4, space="PSUM") as mmps,
            tc.tile_pool(name="tps", bufs=2, space="PSUM") as tps,
        ):
            gexp = _build_routing(nc, const, small, rpsum, dram)

            # resident weights
            w_tiles, a_tiles, lb_tiles, b_tiles = [], [], [], []
            for l, (IN, OUT) in enumerate(DIMS):
                KT = IN // 128
                wl, al = [], []
                for k in range(KT):
                    wt_t = wpool.tile([128, OUT], BF16, tag=f"w{l}_{k}")
                    nc.sync.dma_start(out=wt_t[:], in_=wts[l][k * 128 : (k + 1) * 128, :])
                    wl.append(wt_t)
                    at_t = wpool.tile([128, E * RK], BF16, tag=f"a{l}_{k}")
                    nc.sync.dma_start(out=at_t[:], in_=ats[l][k * 128 : (k + 1) * 128, :])
                    al.append(at_t)
                w_tiles.append(wl)
                a_tiles.append(al)
                lb_t = wpool.tile([E * RK, OUT], BF16, tag=f"lb{l}")
                nc.sync.dma_start(out=lb_t[:], in_=lbs[l][:])
                lb_tiles.append(lb_t)
                b_t = wpool.tile([128, OUT // 128], F32, tag=f"b{l}")
                nc.sync.dma_start(
                    out=b_t[:], in_=biases[l].rearrange("(o p) -> p o", p=128)
                )
                b_tiles.append(b_t)

            # per-token gamma rows via one-hot gather
            gammas = [gpool.tile([E * RK, B_LOC], BF16, tag=f"g{l}", name=f"gamma{l}") for l in range(L)]
            for n in range(NT):
                on_t = onp.tile([M, NB], F32, tag="on")
                nc.sync.dma_start(out=on_t[:], in_=onehot[:, n * NB : (n + 1) * NB])
                for l in range(L):
                    gps = tps.tile([E * RK, NB], F32, tag="tpsum")
                    nc.tensor.matmul(
                        gps[:], gexp[:, l * 32 : (l + 1) * 32], on_t[:],
                        start=True, stop=True,
                    )
                    nc.vector.tensor_copy(gammas[l][:, n * NB : (n + 1) * NB], gps[:])

            # main fused pipeline: per batch-column, all three layers
            for n in range(NT):
                cols = []  # current layer input tiles
                for k in range(D0 // 128):
                    xk = xcolp.tile([128, NB], BF16, tag="xcol")
                    nc.sync.dma_start(
                        out=xk[:], in_=xt[k * 128 : (k + 1) * 128, n * NB : (n + 1) * NB]
                    )
                    cols.append(xk)

                for l, (IN, OUT) in enumerate(DIMS):
                    KT, OT = IN // 128, OUT // 128
                    # LoRA A-side: t = A^T-contract over features, scaled by gamma
                    t_ps = tps.tile([E * RK, NB], F32, tag="tpsum")
                    for k in range(KT):
                        nc.tensor.matmul(
                            t_ps[:], a_tiles[l][k][:], cols[k][:],
                            start=(k == 0), stop=(k == KT - 1),
                        )
                    tw = twp.tile([E * RK, NB], BF16, tag="tw")
                    nc.vector.tensor_mul(
                        tw[:], t_ps[:], gammas[l][:, n * NB : (n + 1) * NB]
                    )

                    nxt = []
                    for o in range(OT):
                        ps = mmps.tile([128, NB], F32, tag="mm")
                        for k in range(KT):
                            nc.tensor.matmul(
                                ps[:], w_tiles[l][k][:, o * 128 : (o + 1) * 128],
                                cols[k][:], start=(k == 0), stop=False,
                            )
                        nc.tensor.matmul(
                            ps[:], lb_tiles[l][:, o * 128 : (o + 1) * 128], tw[:],
                            start=False, stop=True,
                        )
                        if l < 2:
                            pool = h1p if l == 0 else h2p
                            ot = pool.tile([128, NB], BF16, tag=f"h{l + 1}")
                            nc.scalar.activation(
                                ot[:], ps[:], AF.Relu, bias=b_tiles[l][:, o : o + 1]
                            )
                            nxt.append(ot)
                        else:
                            ot = ocp.tile([128, NB], F32, tag="oc")
                            nc.scalar.activation(
                                ot[:], ps[:], AF.Relu, bias=b_tiles[l][:, o : o + 1]
                            )
                            nc.sync.dma_start(
                                out=out_d[o * 128 : (o + 1) * 128, n * NB : (n + 1) * NB],
                                in_=ot[:],
                            )
                    cols = nxt
    return nc


_CACHED = {}


def _get_nc():
    if "nc" not in _CACHED:
        nc = bass.Bass()
        _build(nc)
        _patch_bass_json(nc)
        _CACHED["nc"] = nc
    return _CACHED["nc"]


def kernel(**inputs) -> np.ndarray:
    x = np.asarray(inputs["x"], np.float32)
    ids = np.asarray(inputs["domain_ids"]).astype(np.int64)
    f32 = lambda a: np.ascontiguousarray(np.asarray(a), np.float32)
    bf = lambda a: np.ascontiguousarray(np.asarray(a, np.float32).astype(BF_NP))

    W = [f32(inputs[f"W{i}"]) for i in (1, 2, 3)]
    Bv = [f32(inputs[f"b{i}"]) for i in (1, 2, 3)]
    A = [f32(inputs[f"A{i}"]) for i in (1, 2, 3)]
    Bl = [f32(inputs[f"B{i}"]) for i in (1, 2, 3)]

    dom_emb, layer_pos = f32(inputs["dom_emb"]), f32(inputs["layer_pos"])
    rin = np.concatenate(
        [
            np.broadcast_to(dom_emb[:, None, :], (M, L, H)),
            np.broadcast_to(layer_pos[None, :, :], (M, L, H)),
        ],
        axis=-1,
    ).reshape(M * L, 2 * H).T

    shared = {
        "wi1t": f32(inputs["Wi1"]).T, "wa1t": f32(inputs["Wa1"]).T,
        "bi1v": f32(inputs["bi1"]), "ba1v": f32(inputs["ba1"]),
        "wi2b": np.concatenate([f32(inputs["Wi2"]).T, f32(inputs["bi2"])[None, :]], 0),
        "wa2b": np.concatenate([f32(inputs["Wa2"]).T, f32(inputs["ba2"])[None, :]], 0),
        "gatet": f32(inputs["gate_logits"]).T, "rbt": f32(inputs["R_benefit"]).T,
        "rin": rin,
    }
    shared = {k: f32(v) for k, v in shared.items()}
    for l in range(3):
        shared[f"w{l + 1}t"] = bf(W[l].T)
        shared[f"a{l + 1}t"] = bf(A[l].reshape(E * RK, -1).T)
        shared[f"lb{l + 1}"] = bf(Bl[l].transpose(0, 2, 1).reshape(E * RK, -1))
        shared[f"bias{l + 1}"] = Bv[l]

    in_maps = []
    for i in range(N_CORES):
        sl = slice(i * B_LOC, (i + 1) * B_LOC)
        m = dict(shared)
        m["xt"] = bf(x[sl].T)
        m["onehot"] = np.ascontiguousarray(
            (ids[sl][None, :] == np.arange(M)[:, None]).astype(np.float32)
        )
        in_maps.append(m)

    nc = _get_nc()
    res = run_bass_kernel_spmd(nc, in_maps, core_ids=list(range(N_CORES)))
    return np.concatenate(
        [np.asarray(res.results[i]["out"], np.float32).T for i in range(N_CORES)], axis=0
    )
